# revision 6
# baseline (speedup 1.0000x reference)
"""ChebNet (K=3, 3 layers) GNN on 8 Trainium2 NeuronCores — v2.

Math per layer (L v = -dis*S(dis*v), S = unweighted scatter-add over edges,
dis = rsqrt(clamp(outdeg,1)) masked by outdeg>0):
    out = h@(W0-W2) - dis*v1 + 2*dis*w
    v1  = S((dis h)@W1)          (pass 1, fused)
    u1  = S((dis h)@W2)          (pass 1, fused)
    w   = S(dis^2 * u1)          (pass 2)
Pass 1 gathers the 128-wide concatenation [(dis h)@W1 | (dis h)@W2] per edge
(one 256B bf16 element) and scatters both with a single one-hot matmul.

Perf-critical choices (vs v1):
  * everything in the lap datapath is bf16 (FWL weight loads, 2x DVE modes)
  * one-hot builds are [128,129] bf16 — odd innermost dim keeps the DVE in
    single-port mode so it never takes the shared SBUF port lock that blocks
    GpSimd SWDGE descriptor generation (the v1 bottleneck)
  * dis tables precomputed on host (no on-device degree pass)
  * h is stored transposed (hT); hsT = hT * dis-row via one big TT per layer
  * per-node scalings run on the scalar (ACT) engine, which has its own port
"""

import os
import sys

sys.path.insert(0, "/opt/trn_rl_repo")

import numpy as np
import ml_dtypes
from contextlib import ExitStack

BF = ml_dtypes.bfloat16

_REAL = dict(N=50000, E=800000, NCORES=8, LO=32768, F0=128, F1=64, F2=16)


# ---------------------------------------------------------------- host prep
def _derive(cfg):
    c = dict(cfg)
    c["NPC"] = c["N"] // c["NCORES"]
    c["NT"] = -(-c["NPC"] // 128)
    c["NPAD"] = c["NT"] * 128
    c["NG"] = c["NCORES"] * c["NPAD"]
    c["FW"] = 64   # per-operand feature lanes
    c["FG"] = 128  # gather row width (bf16 -> 256B elements)
    assert c["LO"] <= 32768 and c["NG"] - c["LO"] <= 32768
    assert c["N"] % c["NCORES"] == 0
    return c


def _prep(edge_index, c):
    N, E, NCORES, LO = c["N"], c["E"], c["NCORES"], c["LO"]
    NPC, NT, NPAD = c["NPC"], c["NT"], c["NPAD"]

    src = np.asarray(edge_index[0], dtype=np.int64)
    dst = np.asarray(edge_index[1], dtype=np.int64)
    assert src.shape == (E,) and dst.shape == (E,)
    psrc = (src // NPC) * NPAD + (src % NPC)  # padded global row of src

    cd = dst // NPC
    ld = dst - cd * NPC
    td = ld >> 7
    dl = ld & 127
    hi = (psrc >= LO).astype(np.int64)

    # ---- lap tables: edges grouped by (core, dst-tile, window), src-sorted
    counts = np.zeros((NCORES, NT, 2), np.int64)
    np.add.at(counts, (cd, td, hi), 1)
    Klo = np.maximum(1, -(-counts[:, :, 0].max(0) // 128))
    Khi = np.maximum(1, -(-counts[:, :, 1].max(0) // 128))
    LOFF = np.concatenate([[0], np.cumsum(Klo)]).astype(np.int64)
    HOFF = np.concatenate([[0], np.cumsum(Khi)]).astype(np.int64)
    TLO, THI = int(LOFF[-1]), int(HOFF[-1])

    order = np.lexsort((psrc, hi, td, cd))
    cd_s, td_s, hi_s = cd[order], td[order], hi[order]
    dl_s, psrc_s = dl[order], psrc[order]
    grp = (cd_s * NT + td_s) * 2 + hi_s
    gc = np.bincount(grp, minlength=NCORES * NT * 2)
    gstart = np.concatenate([[0], np.cumsum(gc)])[:-1]
    rank = np.arange(E) - gstart[grp]

    gidx_lo = np.zeros((NCORES, TLO * 128), np.int16)
    gidx_hi = np.zeros((NCORES, THI * 128), np.int16)
    dloc_lo = np.full((NCORES, 128, TLO), -1.0, np.float32)
    dloc_hi = np.full((NCORES, 128, THI), -1.0, np.float32)
    for cc in range(NCORES):
        for h, (gidx, dloc, OFF, base) in enumerate(
            [(gidx_lo, dloc_lo, LOFF, 0), (gidx_hi, dloc_hi, HOFF, LO)]
        ):
            m = (cd_s == cc) & (hi_s == h)
            slot = OFF[td_s[m]] + rank[m] // 128
            part = rank[m] & 127
            gidx[cc, slot * 128 + part] = (psrc_s[m] - base).astype(np.int16)
            dloc[cc, part, slot] = dl_s[m].astype(np.float32)

    # ---- dis tables (host-side degree)
    deg = np.bincount(src, minlength=N).astype(np.float64)
    dis = np.where(deg > 0, 1.0 / np.sqrt(np.maximum(deg, 1.0)), 0.0).astype(
        np.float32
    )
    dis_t = np.zeros((NCORES, 128, NT), np.float32)
    dismt = np.zeros((NCORES, 128, NPAD), np.float32)
    for cc in range(NCORES):
        dcore = np.zeros(NPAD, np.float32)
        dcore[:NPC] = dis[cc * NPC : (cc + 1) * NPC]
        dis_t[cc] = dcore.reshape(NT, 128).T
        dismt[cc] = np.tile(dcore, (128, 1))

    def wrap(a):  # int16 [M*128] -> [128, M*8], idx j at [j%16, j//16], x8 replicated
        return np.tile(a.reshape(-1, 16).T, (8, 1)).copy()

    return dict(
        Klo=Klo, Khi=Khi, LOFF=LOFF, HOFF=HOFF, TLO=TLO, THI=THI,
        gidx_lo=[wrap(gidx_lo[cc]) for cc in range(NCORES)],
        gidx_hi=[wrap(gidx_hi[cc]) for cc in range(NCORES)],
        dloc_lo=dloc_lo, dloc_hi=dloc_hi,
        dis_t=dis_t, dismt=dismt,
    )


# ---------------------------------------------------------------- device build
def _build(c, pp, Fins, use_bias):
    import concourse.bacc as bacc
    import concourse.tile as tile
    from concourse import mybir

    f32, i16, bf16 = mybir.dt.float32, mybir.dt.int16, mybir.dt.bfloat16
    AOT = mybir.AluOpType
    ACT = mybir.ActivationFunctionType
    NT, NPAD, NG, LO = c["NT"], c["NPAD"], c["NG"], c["LO"]
    NCORES, F0, F2, FW, FG = c["NCORES"], c["F0"], c["F2"], c["FW"], c["FG"]
    TLO, THI = pp["TLO"], pp["THI"]
    Klo, Khi = pp["Klo"], pp["Khi"]
    LOFF, HOFF = pp["LOFF"], pp["HOFF"]
    GRPL = int(os.environ.get("KGRPL", "7"))
    CAP = int(os.environ.get("KCAP", "8"))
    groups = [list(range(g * GRPL, min((g + 1) * GRPL, NT))) for g in range(-(-NT // GRPL))]

    nc = bacc.Bacc(num_devices=NCORES)

    xin = nc.dram_tensor("x", [NPAD, F0], f32, kind="ExternalInput")
    gl_d = nc.dram_tensor("gidx_lo", [128, TLO * 8], i16, kind="ExternalInput")
    gh_d = nc.dram_tensor("gidx_hi", [128, THI * 8], i16, kind="ExternalInput")
    dl_d = nc.dram_tensor("dloc_lo", [128, TLO], f32, kind="ExternalInput")
    dh_d = nc.dram_tensor("dloc_hi", [128, THI], f32, kind="ExternalInput")
    iota_d = nc.dram_tensor("iota", [128, 129], bf16, kind="ExternalInput")
    id_d = nc.dram_tensor("identb", [128, 128], bf16, kind="ExternalInput")
    dis_d = nc.dram_tensor("dis", [128, NT], f32, kind="ExternalInput")
    ndis_d = nc.dram_tensor("ndis", [128, NT], f32, kind="ExternalInput")
    dis2_d = nc.dram_tensor("dis2", [128, NT], f32, kind="ExternalInput")
    dis2x_d = nc.dram_tensor("dis2x", [128, NT], f32, kind="ExternalInput")
    dismt_d = nc.dram_tensor("dismt", [128, NPAD], bf16, kind="ExternalInput")
    W_d = {}
    for l in range(3):
        W_d["wcat", l] = nc.dram_tensor(f"wcat{l}", [Fins[l], FG], bf16, kind="ExternalInput")
        W_d["wa", l] = nc.dram_tensor(f"wa{l}", [Fins[l], FW], bf16, kind="ExternalInput")
        if use_bias[l]:
            W_d["br", l] = nc.dram_tensor(f"br{l}", [128, FW], bf16, kind="ExternalInput")
    y_d = nc.dram_tensor("y", [NPAD, F2], f32, kind="ExternalOutput")

    ag_in = [nc.dram_tensor(f"agin{i}", [NPAD, FG], bf16) for i in range(6)]
    ag_out = [nc.dram_tensor(f"agout{i}", [NG, FG], bf16, addr_space="Shared") for i in range(6)]
    agv = [t.rearrange("(t p) f -> p t f", p=128) for t in ag_in]

    xv = xin.rearrange("(t p) f -> p t f", p=128)
    yv = y_d.rearrange("(t p) f -> p t f", p=128)

    with tile.TileContext(nc) as tc, ExitStack() as ctx:
        cst = ctx.enter_context(tc.tile_pool(name="cst", bufs=1))
        big = ctx.enter_context(tc.tile_pool(name="big", bufs=1))
        gp = ctx.enter_context(tc.tile_pool(name="gp", bufs=2))
        ohp = ctx.enter_context(tc.tile_pool(name="ohp", bufs=6))
        smp = ctx.enter_context(tc.tile_pool(name="smp", bufs=6))
        slb = ctx.enter_context(tc.tile_pool(name="slb", bufs=3))
        psA = ctx.enter_context(tc.tile_pool(name="psA", bufs=3, space="PSUM"))
        psD = ctx.enter_context(tc.tile_pool(name="psD", bufs=2, space="PSUM"))
        psT = ctx.enter_context(tc.tile_pool(name="psT", bufs=2, space="PSUM"))

        # ---------------- constants
        iota = cst.tile([128, 129], bf16)
        nc.sync.dma_start(iota[:], iota_d[:])
        identb = cst.tile([128, 128], bf16)
        nc.sync.dma_start(identb[:], id_d[:])
        gl = cst.tile([128, TLO * 8], i16)
        nc.sync.dma_start(gl[:], gl_d[:])
        gh = cst.tile([128, THI * 8], i16)
        nc.sync.dma_start(gh[:], gh_d[:])
        dlo = cst.tile([128, TLO], f32)
        nc.sync.dma_start(dlo[:], dl_d[:])
        dhi = cst.tile([128, THI], f32)
        nc.sync.dma_start(dhi[:], dh_d[:])
        dis = cst.tile([128, NT], f32)
        nc.sync.dma_start(dis[:], dis_d[:])
        ndis = cst.tile([128, NT], f32)
        nc.sync.dma_start(ndis[:], ndis_d[:])
        dis2 = cst.tile([128, NT], f32)
        nc.sync.dma_start(dis2[:], dis2_d[:])
        dis2x = cst.tile([128, NT], f32)
        nc.sync.dma_start(dis2x[:], dis2x_d[:])
        dismt = cst.tile([128, NPAD], bf16)
        nc.sync.dma_start(dismt[:], dismt_d[:])
        Wt = {}
        for k, d in W_d.items():
            Wt[k] = cst.tile([128, d.shape[1]], bf16, name=f"w_{k[0]}_{k[1]}", tag=f"w_{k[0]}_{k[1]}")
            nc.sync.dma_start(Wt[k][: d.shape[0], :], d[:])

        x_sb = big.tile([128, NT, F0], f32, tag="xsb")
        nc.sync.dma_start(x_sb[:], xv[:])

        hbuf = [
            big.tile([128, NT * 128], bf16, name="h0", tag="h0"),
            big.tile([128, NT * 128], bf16, name="h1", tag="h1"),
        ]
        hsT = big.tile([128, NT * 128], bf16, tag="hs")
        v1b = big.tile([128, NT, FW], bf16, tag="v1")
        oab = big.tile([128, NT, FW], bf16, tag="oa")
        ybuf = big.tile([128, NT, F2], f32, tag="yb")

        def gather_group(agi, tl, width):
            a_lo, b_lo = int(LOFF[tl[0]]), int(LOFF[tl[-1] + 1])
            a_hi, b_hi = int(HOFF[tl[0]]), int(HOFF[tl[-1] + 1])
            nlo, nhi = b_lo - a_lo, b_hi - a_hi
            glo = gp.tile([128, nlo, FG], bf16, tag="glo")
            for o in range(0, nlo, CAP):
                n = min(CAP, nlo - o)
                nc.gpsimd.dma_gather(
                    glo[:, o : o + n, :], ag_out[agi][0:LO, :],
                    gl[:, (a_lo + o) * 8 : (a_lo + o + n) * 8],
                    num_idxs=n * 128, num_idxs_reg=n * 128, elem_size=FG,
                )
            ghi_t = gp.tile([128, nhi, FG], bf16, tag="ghi")
            for o in range(0, nhi, CAP):
                n = min(CAP, nhi - o)
                nc.gpsimd.dma_gather(
                    ghi_t[:, o : o + n, :], ag_out[agi][LO:NG, :],
                    gh[:, (a_hi + o) * 8 : (a_hi + o + n) * 8],
                    num_idxs=n * 128, num_idxs_reg=n * 128, elem_size=FG,
                )
            return glo, ghi_t, a_lo, a_hi

        def scatter_tile(t, acc, glo, ghi_t, a_lo, a_hi, width):
            ntot = int(Klo[t]) + int(Khi[t])
            i = 0
            for k in range(int(Klo[t])):
                s = int(LOFF[t]) + k
                oh = ohp.tile([128, 129], bf16, tag="oh")
                nc.vector.tensor_scalar(oh[:], iota[:], dlo[:, s : s + 1], None, AOT.is_equal)
                nc.tensor.matmul(
                    acc, oh[:, 0:128], glo[:, s - a_lo, 0:width],
                    start=(i == 0), stop=(i == ntot - 1),
                )
                i += 1
            for k in range(int(Khi[t])):
                s = int(HOFF[t]) + k
                oh = ohp.tile([128, 129], bf16, tag="oh")
                nc.vector.tensor_scalar(oh[:], iota[:], dhi[:, s : s + 1], None, AOT.is_equal)
                nc.tensor.matmul(
                    acc, oh[:, 0:128], ghi_t[:, s - a_hi, 0:width],
                    start=(i == 0), stop=(i == ntot - 1),
                )
                i += 1

        # ---------------- layers
        for l in range(3):
            Fin = Fins[l]
            hT = hbuf[l % 2]
            hTn = hbuf[(l + 1) % 2]
            agA, agC = 2 * l, 2 * l + 1

            # ---- hT / hsT
            if l == 0:
                for t in range(NT):
                    xb = smp.tile([128, F0], bf16, tag="xb")
                    nc.scalar.copy(xb[:], x_sb[:, t, :])
                    xs = smp.tile([128, F0], bf16, tag="xs")
                    nc.scalar.mul(xs[:], x_sb[:, t, :], dis[:, t : t + 1])
                    p1 = psT.tile([128, 128], bf16, tag="pt")
                    nc.tensor.transpose(p1[:F0, :], xb[:], identb[:])
                    nc.scalar.copy(hT[:F0, t * 128 : (t + 1) * 128], p1[:F0, :])
                    p2 = psT.tile([128, 128], bf16, tag="pt")
                    nc.tensor.transpose(p2[:F0, :], xs[:], identb[:])
                    nc.scalar.copy(hsT[:F0, t * 128 : (t + 1) * 128], p2[:F0, :])
            else:
                nc.vector.tensor_mul(hsT[:Fin, :], hT[:Fin, :], dismt[:Fin, :])

            # ---- dense phase + ag1 input
            for tl in groups:
                p1s = slb.tile([128, GRPL, FG], bf16, tag="p1s")
                for u, t in enumerate(tl):
                    pd = psD.tile([128, FG + FW], f32, tag="pd")
                    nc.tensor.matmul(
                        pd[:, 0:FG], hsT[:Fin, t * 128 : (t + 1) * 128],
                        Wt["wcat", l][:Fin, :], start=True, stop=True,
                    )
                    nc.tensor.matmul(
                        pd[:, FG : FG + FW], hT[:Fin, t * 128 : (t + 1) * 128],
                        Wt["wa", l][:Fin, :], start=True, stop=True,
                    )
                    nc.scalar.copy(p1s[:, u, :], pd[:, 0:FG])
                    nc.scalar.copy(oab[:, t, :], pd[:, FG : FG + FW])
                nc.sync.dma_start(
                    agv[agA][:, tl[0] : tl[0] + len(tl), :], p1s[:, 0 : len(tl), :]
                )
            nc.gpsimd.collective_compute(
                "AllGather", mybir.AluOpType.bypass,
                replica_groups=[list(range(NCORES))],
                ins=[ag_in[agA][:, :]], outs=[ag_out[agA][:, :]],
            )

            # ---- pass 1: [v1 | u1] = S([.@W1 | .@W2])
            for tl in groups:
                glo, ghi_t, a_lo, a_hi = gather_group(agA, tl, FG)
                wops = slb.tile([128, GRPL, FG], bf16, tag="wop")
                for u, t in enumerate(tl):
                    acc = psA.tile([128, FG], f32, tag="acc")
                    scatter_tile(t, acc[:], glo, ghi_t, a_lo, a_hi, FG)
                    nc.scalar.mul(wops[:, u, 0:FW], acc[:, FW:FG], dis2[:, t : t + 1])
                    nc.scalar.copy(v1b[:, t, :], acc[:, 0:FW])
                    nc.scalar.copy(wops[:, u, FW:FG], acc[:, 0:FW])  # init filler (never consumed)
                nc.sync.dma_start(
                    agv[agC][:, tl[0] : tl[0] + len(tl), :], wops[:, 0 : len(tl), :]
                )
            nc.gpsimd.collective_compute(
                "AllGather", mybir.AluOpType.bypass,
                replica_groups=[list(range(NCORES))],
                ins=[ag_in[agC][:, :]], outs=[ag_out[agC][:, :]],
            )

            # ---- pass 2: w = S(dis^2 u1); out = Oa - dis v1 + 2 dis w
            for tl in groups:
                glo, ghi_t, a_lo, a_hi = gather_group(agC, tl, FW)
                for u, t in enumerate(tl):
                    accw = psA.tile([128, FG], f32, tag="acc")
                    acc2 = accw[:, 0:FW]
                    scatter_tile(t, acc2, glo, ghi_t, a_lo, a_hi, FW)
                    t2 = smp.tile([128, FW], bf16, tag="t2")
                    nc.scalar.mul(t2[:], acc2, dis2x[:, t : t + 1])
                    t1 = smp.tile([128, FW], bf16, tag="t1")
                    nc.scalar.mul(t1[:], v1b[:, t, :], ndis[:, t : t + 1])
                    s = smp.tile([128, FW], bf16, tag="s")
                    nc.vector.tensor_add(s[:], t1[:], t2[:])
                    pre = smp.tile([128, FW], bf16, tag="pre")
                    nc.vector.tensor_add(pre[:], s[:], oab[:, t, :])
                    if use_bias[l]:
                        pre2 = smp.tile([128, FW], bf16, tag="pre2")
                        nc.vector.tensor_add(pre2[:], pre[:], Wt["br", l][:, :])
                        pre = pre2
                    if l < 2:
                        hr = smp.tile([128, FW], bf16, tag="hr")
                        nc.scalar.activation(hr[:], pre[:], ACT.Relu)
                        pt = psT.tile([128, 128], bf16, tag="pt")
                        nc.tensor.transpose(pt[:FW, :], hr[:], identb[:])
                        nc.scalar.copy(hTn[:FW, t * 128 : (t + 1) * 128], pt[:FW, :])
                    else:
                        nc.scalar.copy(ybuf[:, t, :], pre[:, 0:F2])

        nc.sync.dma_start(yv[:], ybuf[:])

    nc.compile()
    return nc


# ---------------------------------------------------------------- entry
def _run(x, edge_index, Ws, bs, cfg=None, trace=False):
    from concourse.bass_utils import run_bass_kernel_spmd

    c = _derive(cfg or _REAL)
    N, NCORES, NPC, NPAD = c["N"], c["NCORES"], c["NPC"], c["NPAD"]
    F0, F2, FW, FG = c["F0"], c["F2"], c["FW"], c["FG"]

    x = np.ascontiguousarray(np.asarray(x, dtype=np.float32))
    pp = _prep(edge_index, c)

    Fins = [F0, c["F1"], c["F1"]]
    use_bias = [bool(np.any(b)) for b in bs]
    nc = _build(c, pp, Fins, use_bias)

    iota = np.tile(np.arange(129, dtype=np.float32), (128, 1)).astype(BF)
    identb = np.eye(128, dtype=np.float32).astype(BF)

    def pad(w, fin, fw):
        out = np.zeros((fin, fw), np.float32)
        out[: w.shape[0], : w.shape[1]] = w
        return out

    base = {"iota": iota, "identb": identb}
    for l in range(3):
        W = np.asarray(Ws[l], dtype=np.float32)
        wcat = np.concatenate(
            [pad(W[1], Fins[l], FW), pad(W[2], Fins[l], FW)], axis=1
        )
        base[f"wcat{l}"] = wcat.astype(BF)
        base[f"wa{l}"] = pad(W[0] - W[2], Fins[l], FW).astype(BF)
        if use_bias[l]:
            br = np.zeros((128, FW), np.float32)
            br[:, : bs[l].shape[0]] = np.asarray(bs[l], np.float32)
            base[f"br{l}"] = br.astype(BF)

    in_maps = []
    for cc in range(NCORES):
        xl = np.zeros((NPAD, F0), np.float32)
        xl[:NPC] = x[cc * NPC : (cc + 1) * NPC]
        dt = pp["dis_t"][cc]
        in_maps.append(
            dict(
                base,
                x=xl,
                gidx_lo=pp["gidx_lo"][cc],
                gidx_hi=pp["gidx_hi"][cc],
                dloc_lo=np.ascontiguousarray(pp["dloc_lo"][cc]),
                dloc_hi=np.ascontiguousarray(pp["dloc_hi"][cc]),
                dis=np.ascontiguousarray(dt),
                ndis=np.ascontiguousarray(-dt),
                dis2=np.ascontiguousarray(dt * dt),
                dis2x=np.ascontiguousarray(2.0 * dt),
                dismt=np.ascontiguousarray(pp["dismt"][cc]).astype(BF),
            )
        )

    res = run_bass_kernel_spmd(nc, in_maps, core_ids=list(range(NCORES)), trace=trace)
    out = np.concatenate([res.results[cc]["y"][:NPC] for cc in range(NCORES)], axis=0)
    return np.ascontiguousarray(out[:, :F2], dtype=np.float32), res


def kernel(x, edge_index, W1, b1, Wm, bm, W2, b2):
    out, _ = _run(
        np.asarray(x), np.asarray(edge_index),
        [np.asarray(W1), np.asarray(Wm), np.asarray(W2)],
        [np.asarray(b1), np.asarray(bm), np.asarray(b2)],
    )
    return out
